# revision 5
# baseline (speedup 1.0000x reference)
"""Block-sparse linear kernel for Trainium2 (8 NeuronCores).

Computes: mask = mean|x| per 64x64 block > 0.798; out = (x*mask) @ weight
for x [4096,4096] f32, weight [4096,11008] f32 -> out [4096,11008] f32.

Strategy:
- Weight column-sharded across 8 cores (1376 cols each); x replicated.
  Each core computes its output shard; host concatenates. No collectives.
- Block sparsity (~48% live) exploited by skipping dead 64x64 x-blocks:
  the host computes the mask, packs a compacted bf16 "lhsT stream" of
  transposed live x blocks, and emits a mask-specialized schedule
  (recompiled per input; the NEFF cache makes repeat calls fast).
- PE packing: m-blocks are paired into M=128 matmul tiles (K=64 each);
  a k-block parity assignment (balanced host-side local search) places
  each k-block in array row-half 0 or 1, so two K=64 matmuls (one per
  parity) run concurrently. TRN2 crashes if one PSUM accumulation group
  mixes row offsets, so the parities accumulate in separate PSUM banks,
  summed at drain (ACT copy + DVE add). m-pairs are matched (greedy +
  2-opt) to maximize live-set overlap; a dead half-block within a live
  (pair, k) tile is zero-padded in the stream.
- bf16 inputs (fp32 PSUM accumulation): fp32 matmul is 4x slower and
  crashes TRN2 here; float32r would double DMA and SBUF instead.
"""

import numpy as np
import ml_dtypes

import concourse.bacc as bacc
import concourse.mybir as mybir
import concourse.tile as tile
from concourse.bass_utils import run_bass_kernel_spmd

M = 4096
K = 4096
N = 11008
B = 64            # sparsity block
NB = M // B       # 64 blocks per dim
NCORES = 8
NSH = N // NCORES  # 1376 output cols per core
THRES = 0.798
CHUNKS = [(0, 512), (512, 512), (1024, 352)]  # N-chunks per psum bank
G = 16            # steps per x-stream DMA group
BF16 = mybir.dt.bfloat16
F32 = mybir.dt.float32


def _block_mask(x):
    xb = np.abs(x.reshape(NB, B, NB, B))
    bm = xb.mean(axis=(1, 3), dtype=np.float64)
    return bm > THRES


def _parity_assign(mask):
    """Balanced parity s[NB] in {0,1} minimizing sum_m |ev_m - od_m|.
    Deterministic local search."""
    Mi = mask.astype(np.int32)
    rng = np.random.default_rng(1234)
    best_sig, best_c = None, 1 << 30
    for _ in range(6):
        sig = np.array([1] * (NB // 2) + [-1] * (NB // 2))
        rng.shuffle(sig)
        improved = True
        while improved:
            improved = False
            d = Mi @ sig
            cur = np.abs(d).sum()
            pos = np.where(sig == 1)[0]
            neg = np.where(sig == -1)[0]
            bestswap, bestdelta = None, 0
            for i in pos:
                for j in neg:
                    nd = d - 2 * Mi[:, i] + 2 * Mi[:, j]
                    delta = np.abs(nd).sum() - cur
                    if delta < bestdelta:
                        bestdelta, bestswap = delta, (i, j)
            if bestswap is not None:
                i, j = bestswap
                sig[i], sig[j] = -1, 1
                improved = True
        c = np.abs(Mi @ sig).sum()
        if c < best_c:
            best_c, best_sig = c, sig.copy()
    return (best_sig == -1).astype(np.int8)  # 1 = odd (array rows 64:128)


def _match_pairs(mask, par):
    """Pair m-blocks to minimize total streamed slots (sum of live-set
    unions). PE wall time is proportional to this. Blossom matching when
    networkx is importable, greedy + 2-opt otherwise."""
    def paircost(i, j):
        return int((mask[i] | mask[j]).sum())

    try:
        import networkx as nx
        g = nx.Graph()
        big = 2 * NB
        for i in range(NB):
            for j in range(i + 1, NB):
                g.add_edge(i, j, weight=big - paircost(i, j))
        mate = nx.algorithms.matching.max_weight_matching(g, maxcardinality=True)
        if len(mate) == NB // 2:
            return [(int(a), int(b)) for a, b in mate]
    except Exception:
        pass

    order = sorted(range(NB), key=lambda i: -int(mask[i].sum()))
    used, pairs = set(), []
    for i in order:
        if i in used:
            continue
        best, bc = None, 1 << 30
        for j in order:
            if j == i or j in used:
                continue
            c = paircost(i, j)
            if c < bc:
                bc, best = c, j
        used.add(i)
        used.add(best)
        pairs.append([i, best])
    improved, it = True, 0
    while improved and it < 100:
        improved = False
        it += 1
        for a in range(len(pairs)):
            for b in range(a + 1, len(pairs)):
                i1, j1 = pairs[a]
                i2, j2 = pairs[b]
                cur = paircost(i1, j1) + paircost(i2, j2)
                for (n1, n2) in (((i1, i2), (j1, j2)), ((i1, j2), (j1, i2))):
                    c = paircost(*n1) + paircost(*n2)
                    if c < cur:
                        pairs[a] = list(n1)
                        pairs[b] = list(n2)
                        cur = c
                        improved = True
    return [(int(a), int(b)) for a, b in pairs]


def _refine_parity(mask, par, pairs):
    """Post-matching parity refinement on the pair-union matrix: global
    even/odd slot balance first (PE row-halves pipeline across pairs),
    per-pair balance second."""
    u = np.stack([mask[a] | mask[b] for a, b in pairs]).astype(np.int64)
    sig = np.where(par == 0, 1, -1).astype(np.int64)

    def cost(s):
        d = u @ s
        return abs(int(d.sum())) * 1000 + int(np.abs(d).sum())

    improved = True
    while improved:
        improved = False
        cur = cost(sig)
        pos = np.where(sig == 1)[0]
        neg = np.where(sig == -1)[0]
        bestswap, bestc = None, cur
        for i in pos:
            for j in neg:
                sig[i], sig[j] = -1, 1
                c = cost(sig)
                sig[i], sig[j] = 1, -1
                if c < bestc:
                    bestc, bestswap = c, (i, j)
        if bestswap is not None:
            i, j = bestswap
            sig[i], sig[j] = -1, 1
            improved = True
    return (sig == -1).astype(np.int8)


def _schedule(mask, par):
    """Pairing + per-pair parity queues (union of the two m-blocks' live
    sets) + W tile layout."""
    evens = [b for b in range(NB) if par[b] == 0]
    odds = [b for b in range(NB) if par[b] == 1]
    assert len(evens) == len(odds) == NB // 2
    wloc = {}
    for t in range(NB // 2):
        wloc[evens[t]] = (t, 0)
        wloc[odds[t]] = (t, 1)
    pairs = _match_pairs(mask, par)
    par = _refine_parity(mask, par, pairs)
    evens = [b for b in range(NB) if par[b] == 0]
    odds = [b for b in range(NB) if par[b] == 1]
    wloc = {}
    for t in range(NB // 2):
        wloc[evens[t]] = (t, 0)
        wloc[odds[t]] = (t, 1)
    sched = []
    for mA, mB in pairs:
        u = mask[mA] | mask[mB]
        qE = [b for b in range(NB) if u[b] and par[b] == 0]
        qO = [b for b in range(NB) if u[b] and par[b] == 1]
        steps = max(len(qE), len(qO))
        sched.append({"mA": mA, "mB": mB, "qE": qE, "qO": qO, "steps": steps})
    total_steps = sum(p["steps"] for p in sched)
    return sched, wloc, total_steps


def _pack_stream(x, mask, sched, total_steps):
    """bf16 lhsT stream [128, S_pad*128]: per step one [128,128] tile:
    rows 0:64 = even k-block's xT (cols 0:64 -> mA, 64:128 -> mB),
    rows 64:128 = odd k-block's xT. Dead half-blocks stay zero."""
    s_pad = ((total_steps + G - 1) // G) * G
    xs = np.zeros((128, s_pad * 128), dtype=np.float32)
    gs = 0
    for p in sched:
        mm = (p["mA"], p["mB"])
        for s in range(p["steps"]):
            col = gs * 128
            for r, queue in ((0, p["qE"]), (1, p["qO"])):
                if s < len(queue):
                    b = queue[s]
                    for h in (0, 1):
                        if mask[mm[h], b]:
                            blk = x[mm[h] * B:(mm[h] + 1) * B, b * B:(b + 1) * B]
                            xs[64 * r:64 * r + 64,
                               col + 64 * h:col + 64 * h + 64] = blk.T
            gs += 1
    assert gs == total_steps
    return xs.astype(ml_dtypes.bfloat16), s_pad


def _w_row_index(wloc):
    idx = np.empty(K, dtype=np.int64)
    for b, (t, r) in wloc.items():
        idx[128 * t + 64 * r: 128 * t + 64 * r + 64] = np.arange(b * B, (b + 1) * B)
    return idx


def _build(sched, wloc, s_pad, reps=1):
    nc = bacc.Bacc()
    xs_d = nc.declare_dram_parameter("xs", [128, s_pad * 128], BF16, isOutput=False)
    w_d = nc.declare_dram_parameter("w", [K, NSH], BF16, isOutput=False)
    out_d = nc.declare_dram_parameter("out", [M, NSH], F32, isOutput=True)

    with tile.TileContext(nc) as tc:
        with (
            tc.tile_pool(name="wp", bufs=1) as wp,
            tc.tile_pool(name="xp", bufs=6) as xp,
            tc.tile_pool(name="dp", bufs=3) as dp,
            tc.tile_pool(name="sp", bufs=2) as sp,
            tc.tile_pool(name="pse", bufs=3, space="PSUM") as ppe,
            tc.tile_pool(name="pso", bufs=3, space="PSUM") as ppo,
        ):
            wts = []
            for t in range(NB // 2):
                wt = wp.tile([128, NSH], BF16, tag=f"w{t}")
                nc.sync.dma_start(wt[:], w_d[128 * t:128 * (t + 1), :])
                wts.append(wt)

            for _ in range(reps):
                xg_tiles = {}
                gs_base = 0
                for p in sched:
                    qE, qO, steps = p["qE"], p["qO"], p["steps"]
                    for g in range(gs_base // G, (gs_base + steps + G - 1) // G):
                        if g not in xg_tiles:
                            xg = xp.tile([128, G * 128], BF16, tag="xg")
                            nc.sync.dma_start(
                                xg[:], xs_d[:, g * G * 128:(g + 1) * G * 128])
                            xg_tiles[g] = xg
                    stage = sp.tile([128, NSH], F32, tag="stage")
                    for (c0, cw) in CHUNKS:
                        ps_e = ppe.tile([128, 512], F32, tag="pse")
                        ps_o = ppo.tile([128, 512], F32, tag="pso")
                        for s in range(steps):
                            gs = gs_base + s
                            xg = xg_tiles[gs // G]
                            col = (gs % G) * 128
                            for r, queue, ps in ((0, qE, ps_e), (1, qO, ps_o)):
                                if s >= len(queue):
                                    continue
                                b = queue[s]
                                t, rr = wloc[b]
                                assert rr == r
                                nc.tensor.matmul(
                                    ps[:, :cw],
                                    lhsT=xg[64 * r:64 * r + 64, col:col + 128],
                                    rhs=wts[t][64 * r:64 * r + 64, c0:c0 + cw],
                                    start=(s == 0),
                                    stop=(s == len(queue) - 1),
                                    skip_group_check=True,
                                )
                        # drain this chunk
                        he, ho = len(qE) > 0, len(qO) > 0
                        dst = stage[:, c0:c0 + cw]
                        if he and ho:
                            tmp = dp.tile([128, 512], F32, tag="tmp")
                            nc.scalar.copy(tmp[:, :cw], ps_o[:, :cw])
                            nc.vector.tensor_tensor(
                                dst, ps_e[:, :cw], tmp[:, :cw],
                                mybir.AluOpType.add)
                        elif he:
                            nc.vector.tensor_copy(dst, ps_e[:, :cw])
                        elif ho:
                            nc.vector.tensor_copy(dst, ps_o[:, :cw])
                        else:
                            nc.vector.memset(dst, 0.0)
                    nc.sync.dma_start(
                        out_d[p["mA"] * B:(p["mA"] + 1) * B, :], stage[0:64, :])
                    nc.sync.dma_start(
                        out_d[p["mB"] * B:(p["mB"] + 1) * B, :], stage[64:128, :])
                    gs_base += steps
    nc.compile()
    return nc


def _prepare(x, weight, reps=1):
    x = np.ascontiguousarray(np.asarray(x, dtype=np.float32))
    weight = np.ascontiguousarray(np.asarray(weight, dtype=np.float32))
    mask = _block_mask(x)
    par = _parity_assign(mask)
    sched, wloc, total_steps = _schedule(mask, par)
    xs, s_pad = _pack_stream(x, mask, sched, total_steps)
    widx = _w_row_index(wloc)
    wperm = weight[widx].astype(ml_dtypes.bfloat16)
    in_maps = [
        {"xs": xs, "w": np.ascontiguousarray(wperm[:, c * NSH:(c + 1) * NSH])}
        for c in range(NCORES)
    ]
    nc = _build(sched, wloc, s_pad, reps=reps)
    return nc, in_maps


def kernel(x, weight):
    nc, in_maps = _prepare(x, weight)
    res = run_bass_kernel_spmd(nc, in_maps, core_ids=list(range(NCORES)))
    out = np.concatenate([res.results[c]["out"] for c in range(NCORES)], axis=1)
    return np.ascontiguousarray(out)
